# revision 9
# baseline (speedup 1.0000x reference)
# Causal self-attention (B=4, T=2048, C=1024, H=16) on 8 TRN2 NeuronCores.
#
# Sharding: core = 2*b + g  (b in 0..3 data-parallel over batch,
# g in 0..1 tensor-parallel over head halves: 8 heads per core).
# Each core gets x[b] and the column slice of Wq/Wk/Wv (cols g*512..) and the
# row slice of Wp (rows g*512..), computes a partial y[b] = attn_g(x[b]) @ Wp_g,
# and the host sums the two partials per batch (the "all-reduce") and adds the
# bias terms (bv @ wp + bp), which commute exactly through softmax-normalized
# attention.
#
# On-chip layout is fully transposed ("channels on partitions"):
#   xT[c, t]  -> QT/KT[d_local, t] (bf16), V[t, d_local] (bf16, +ones col)
#   S^T[k, q] = KT_tile.T @ QT_chunk       (bf16 matmul, k on partitions)
#   P^T = exp(scale*S^T + causal mask)     (ScalarE, no-max softmax, -4 shift)
#   O^T_aug[65, q] = V_aug.T @ P^T         (row 64 accumulates the denominator)
#   O^T_norm = O^T * bcast(1/den)          (gpsimd partition_broadcast + DVE)
#   y[q, c] = sum_h O^T_norm_h.T @ Wp_h    (f32r matmuls, natural output layout)

import math

import numpy as np

B, T, C, H = 4, 2048, 1024, 16
D = 64
NCORES = 8
HC = 8          # heads per core
CL = HC * D     # 512 local channels
QCH = 512       # q chunk
KT = 128        # k tile
NTT = T // 128  # 16 t-tiles
NCH = T // QCH  # 4 chunks
SCALE = 1.0 / math.sqrt(D)
EXP_SHIFT = -4.0
MASK_VAL = -1e30

_CACHE = {}


def _build_bass():
    import concourse.tile as tile
    from concourse import bacc, mybir
    from concourse.masks import make_identity

    dt = mybir.dt
    f32 = dt.float32
    f32r = dt.float32r
    bf16 = dt.bfloat16

    nc = bacc.Bacc(None, target_bir_lowering=False)

    x_d = nc.declare_dram_parameter("x", [T, C], f32, isOutput=False)
    wq_d = nc.declare_dram_parameter("wq", [C, CL], f32, isOutput=False)
    wk_d = nc.declare_dram_parameter("wk", [C, CL], f32, isOutput=False)
    wv_d = nc.declare_dram_parameter("wv", [C, CL], f32, isOutput=False)
    wp_d = nc.declare_dram_parameter("wp", [CL, C], f32, isOutput=False)
    bq_d = nc.declare_dram_parameter("bq", [CL], f32, isOutput=False)
    bk_d = nc.declare_dram_parameter("bk", [CL], f32, isOutput=False)
    out_d = nc.declare_dram_parameter("out", [T, C], f32, isOutput=True)

    NCT = C // 128  # 8 c-tiles
    NM = CL // 128  # 4 dloc-tiles

    with tile.TileContext(nc) as tc:
        with (
            tc.tile_pool(name="const", bufs=1) as constp,
            tc.tile_pool(name="persist", bufs=1) as pers,
            tc.tile_pool(name="ppool", bufs=4, space="PSUM") as ppool,
        ):
            # ---- constants built on-chip ----
            ident = constp.tile([128, 128], f32, tag="ident")
            make_identity(nc, ident[:])

            # master causal mask [128, 640]: master[i, jj] = 0 if jj >= i+512
            # else MASK_VAL.  For a diagonal k-tile with offset off = k0-q0,
            # slicing cols [512-off : 640] gives an additive mask over
            # q-columns [0, off+128) with exactly the causal pattern.
            mask = constp.tile([128, 640], f32, tag="mask")
            nc.gpsimd.memset(mask[:], 0.0)
            nc.gpsimd.affine_select(
                out=mask[:],
                in_=mask[:],
                compare_op=mybir.AluOpType.is_ge,
                fill=MASK_VAL,
                base=-512,
                pattern=[[1, 640]],
                channel_multiplier=-1,
            )

            eshift = constp.tile([128, 1], f32, tag="eshift")
            nc.gpsimd.memset(eshift[:], EXP_SHIFT)

            bq_sb = constp.tile([128, NM], f32, tag="bq")
            bk_sb = constp.tile([128, NM], f32, tag="bk")
            nc.sync.dma_start(bq_sb[:], bq_d.rearrange("(m p) -> p m", p=128))
            nc.sync.dma_start(bk_sb[:], bk_d.rearrange("(m p) -> p m", p=128))

            # ---- persistent tensors ----
            qt = [pers.tile([128, T], bf16, tag=f"qt{m}", name=f"qt{m}") for m in range(NM)]
            kt = [pers.tile([128, T], bf16, tag=f"kt{m}", name=f"kt{m}") for m in range(NM)]
            vt = [pers.tile([128, HC * 65], bf16, tag=f"vt{t}", name=f"vt{t}") for t in range(NTT)]
            wp8 = [pers.tile([64, C], f32r, tag=f"wp{h}", name=f"wp{h}") for h in range(HC)]
            for h in range(HC):
                wptmp = pers.tile([64, C], f32, tag="wptmp", bufs=2, name="wptmp")
                nc.sync.dma_start(wptmp[:], wp_d[h * 64:(h + 1) * 64, :])
                nc.vector.tensor_copy(wp8[h][:], wptmp[:])
            for t in range(NTT):
                # ones column per head for the fused denominator row
                nc.gpsimd.memset(
                    vt[t].rearrange("p (h e) -> p h e", e=65)[:, :, 64:65], 1.0
                )

            # ---- phase A: x transpose + projections, streamed by t-chunk ----
            with (
                tc.tile_pool(name="wpool", bufs=1) as wpool,
                tc.tile_pool(name="xpool", bufs=3) as xpool,
                tc.tile_pool(name="xtc", bufs=16) as xtcp,
                tc.tile_pool(name="tpp", bufs=2, space="PSUM") as tpp,
            ):
                wq_sb = [wpool.tile([128, CL], f32r, tag=f"wq{c}", name=f"wq{c}") for c in range(NCT)]
                wk_sb = [wpool.tile([128, CL], f32r, tag=f"wk{c}", name=f"wk{c}") for c in range(NCT)]
                wv_sb = [wpool.tile([128, CL], f32r, tag=f"wv{c}", name=f"wv{c}") for c in range(NCT)]
                for c in range(NCT):
                    for w_sb, w_d in ((wq_sb, wq_d), (wk_sb, wk_d), (wv_sb, wv_d)):
                        wtmp = xpool.tile([128, CL], f32, tag="wtmp", name="wtmp")
                        nc.sync.dma_start(wtmp[:], w_d[c * 128:(c + 1) * 128, :])
                        nc.vector.tensor_copy(w_sb[c][:], wtmp[:])

                for tch in range(NCH):
                    # load 4 natural x t-tiles, transpose into xT chunk
                    xtc = [xtcp.tile([128, QCH], f32r, tag="xtc", name="xtc") for _ in range(NCT)]
                    for tt in range(4):
                        t0 = tch * QCH + tt * 128
                        xn = xpool.tile([128, C], f32, tag="xn")
                        nc.sync.dma_start(xn[:], x_d[t0:t0 + 128, :])
                        for c in range(NCT):
                            tp = tpp.tile([128, 128], f32, tag="tp")
                            nc.tensor.transpose(
                                tp[:], xn[:, c * 128:(c + 1) * 128], ident[:]
                            )
                            nc.vector.tensor_copy(
                                xtc[c][:, tt * 128:(tt + 1) * 128], tp[:]
                            )

                    # QT / KT: out[dloc_tile, t_chunk] = sum_c wq_c.T @ xT_c
                    for dst, w_sb, b_sb in (
                        (qt, wq_sb, bq_sb),
                        (kt, wk_sb, bk_sb),
                    ):
                        for m in range(NM):
                            ps = ppool.tile([128, QCH], f32, tag="ps")
                            for c in range(NCT):
                                nc.tensor.matmul(
                                    ps[:],
                                    w_sb[c][:, m * 128:(m + 1) * 128],
                                    xtc[c][:],
                                    start=(c == 0),
                                    stop=(c == NCT - 1),
                                )
                            nc.scalar.activation(
                                dst[m][:, tch * QCH:(tch + 1) * QCH],
                                ps[:],
                                mybir.ActivationFunctionType.Identity,
                                bias=b_sb[:, m:m + 1],
                                scale=1.0,
                            )

                    # V: out[t_tile, dloc] = sum_c xT_c_tile.T @ wv_c
                    for tt in range(4):
                        ps = ppool.tile([128, CL], f32, tag="ps")
                        for c in range(NCT):
                            nc.tensor.matmul(
                                ps[:],
                                xtc[c][:, tt * 128:(tt + 1) * 128],
                                wv_sb[c][:],
                                start=(c == 0),
                                stop=(c == NCT - 1),
                            )
                        vtile = vt[tch * 4 + tt]
                        nc.vector.tensor_copy(
                            vtile.rearrange("p (h e) -> p h e", e=65)[:, :, 0:64],
                            ps.rearrange("p (h d) -> p h d", d=64)[:],
                        )

            # ---- phase B: attention + output projection, per q-chunk ----
            with (
                tc.tile_pool(name="stp", bufs=2, space="PSUM") as stp,
                tc.tile_pool(name="otp", bufs=2, space="PSUM") as otp,
                tc.tile_pool(name="ptp", bufs=4) as ptp,
                tc.tile_pool(name="denp", bufs=4) as denp,
                tc.tile_pool(name="bcp", bufs=4) as bcp,
                tc.tile_pool(name="otn", bufs=10) as otnp,
                tc.tile_pool(name="yp", bufs=4) as ypool,
            ):
                for qc in range(NCH):
                    nkt = 4 * (qc + 1)
                    otn = []
                    for pair in range(HC // 2):
                        ot_ps = []
                        for h2 in range(2):
                            h = pair * 2 + h2
                            mt = h // 2
                            r0 = (h % 2) * 64
                            ot = otp.tile([65, QCH], f32, tag="ot")
                            ot_ps.append(ot)
                            for k in range(nkt):
                                st = stp.tile([128, QCH], f32, tag="st")
                                nc.tensor.matmul(
                                    st[:],
                                    kt[mt][r0:r0 + 64, k * 128:(k + 1) * 128],
                                    qt[mt][r0:r0 + 64, qc * QCH:(qc + 1) * QCH],
                                    start=True,
                                    stop=True,
                                )
                                off = (k - 4 * qc) * 128
                                if off >= 0:
                                    nc.vector.tensor_add(
                                        st[:, 0:off + 128],
                                        st[:, 0:off + 128],
                                        mask[:, 512 - off:640],
                                    )
                                pt = ptp.tile([128, QCH], bf16, tag="pt")
                                nc.scalar.activation(
                                    pt[:],
                                    st[:],
                                    mybir.ActivationFunctionType.Exp,
                                    bias=eshift[:],
                                    scale=SCALE,
                                )
                                nc.tensor.matmul(
                                    ot[:],
                                    vt[k][:, h * 65:(h + 1) * 65],
                                    pt[:],
                                    start=(k == 0),
                                    stop=(k == nkt - 1),
                                )
                        # normalize the pair: den lives at psum partition 64;
                        # hw partition_broadcast only reads partition 0, so
                        # bounce the row to partition 0 with a tiny sbuf DMA.
                        for h2 in range(2):
                            den = denp.tile([65, QCH], f32, tag="den")
                            nc.scalar.copy(den[64:65, :], ot_ps[h2][64:65, :])
                            den0 = denp.tile([1, QCH], f32, tag="den0")
                            nc.sync.dma_start(den0[:], den[64:65, :])
                            rec = denp.tile([1, QCH], f32, tag="rec")
                            nc.vector.reciprocal(rec[:], den0[:])
                            bt = bcp.tile([64, QCH], f32, tag="bt")
                            nc.gpsimd.partition_broadcast(bt[:], rec[0:1, :])
                            on = otnp.tile([64, QCH], f32r, tag="otn")
                            nc.vector.tensor_mul(on[:], ot_ps[h2][0:64, :], bt[:])
                            otn.append(on)

                    # projection: y[q, c_out] = sum_h otn_h.T @ wp_h
                    for qs in range(4):
                        q0 = qc * QCH + qs * 128
                        for half in range(2):
                            yps = ppool.tile([128, 512], f32, tag="ps")
                            for h in range(HC):
                                nc.tensor.matmul(
                                    yps[:],
                                    otn[h][:, qs * 128:(qs + 1) * 128],
                                    wp8[h][:, half * 512:(half + 1) * 512],
                                    start=(h == 0),
                                    stop=(h == HC - 1),
                                )
                            ysb = ypool.tile([128, 512], f32, tag="ysb")
                            nc.vector.tensor_copy(ysb[:], yps[:])
                            nc.sync.dma_start(
                                out_d[q0:q0 + 128, half * 512:(half + 1) * 512],
                                ysb[:],
                            )

    return nc


def _get_nc():
    if "nc" not in _CACHE:
        nc = _build_bass()
        nc.compile()
        _CACHE["nc"] = nc
    return _CACHE["nc"]


def _make_in_maps(x, wq, bq, wk, bk, wv, bv, wp, bp):
    in_maps = []
    for core in range(NCORES):
        b, g = core // 2, core % 2
        cs = slice(g * CL, (g + 1) * CL)
        in_maps.append(
            {
                "x": np.ascontiguousarray(x[b], dtype=np.float32),
                "wq": np.ascontiguousarray(wq[:, cs], dtype=np.float32),
                "wk": np.ascontiguousarray(wk[:, cs], dtype=np.float32),
                "wv": np.ascontiguousarray(wv[:, cs], dtype=np.float32),
                "wp": np.ascontiguousarray(wp[cs, :], dtype=np.float32),
                "bq": np.ascontiguousarray(bq[cs], dtype=np.float32),
                "bk": np.ascontiguousarray(bk[cs], dtype=np.float32),
            }
        )
    return in_maps


def kernel(x, wq, bq, wk, bk, wv, bv, wp, bp, _trace=False):
    from concourse.bass_utils import run_bass_kernel_spmd

    x = np.asarray(x, dtype=np.float32)
    nc = _get_nc()
    in_maps = _make_in_maps(x, wq, bq, wk, bk, wv, bv, wp, bp)
    res = run_bass_kernel_spmd(
        nc, in_maps, core_ids=list(range(NCORES)), trace=_trace
    )
    _CACHE["last_results"] = res

    y = np.zeros((B, T, C), dtype=np.float32)
    for core in range(NCORES):
        b = core // 2
        y[b] += res.results[core]["out"]
    # bias terms commute through normalized attention: y += bv @ wp + bp
    const = np.asarray(bv, np.float32) @ np.asarray(wp, np.float32) + np.asarray(
        bp, np.float32
    )
    y += const[None, None, :]
    return y


# revision 10
# speedup vs baseline: 1.1509x; 1.1509x over previous
# Causal self-attention (B=4, T=2048, C=1024, H=16) on 8 TRN2 NeuronCores.
#
# Sharding: core = 2*b + g  (b in 0..3 data-parallel over batch,
# g in 0..1 tensor-parallel over head halves: 8 heads per core).
# Each core gets x[b] and the column slice of Wq/Wk/Wv (cols g*512..) and the
# row slice of Wp (rows g*512..), computes a partial y[b] = attn_g(x[b]) @ Wp_g,
# and the host sums the two partials per batch (the "all-reduce") and adds the
# bias terms (bv @ wp + bp), which commute exactly through softmax-normalized
# attention.
#
# On-chip layout is fully transposed ("channels on partitions"):
#   xT[c, t]  -> QT/KT[d_local, t] (bf16), V[t, d_local] (bf16, +ones col)
#   S^T[k, q] = KT_tile.T @ QT_chunk       (bf16, k on partitions; the two
#              heads of a pair go to the two halves of one [128,1024] psum)
#   P^T = exp(scale*S^T + causal mask)     (one 3D ScalarE exp per pair-tile,
#              no-max softmax with constant -4 shift; masked-left columns
#              skipped and zeroed by a gpsimd memset instead)
#   O^T_aug[65, q] = V_aug.T @ P^T         (row 64 accumulates the denominator)
#   O^T_norm = O^T * bcast(1/den)          (den row -> partition 0 via sbuf
#              DMA, then gpsimd partition_broadcast, then one DVE mul)
#   y[q, c] = sum_h O^T_norm_h.T @ Wp_h    (natural output layout, no final
#              transpose)
#
# Projection work for t-chunk i+1 is emitted between attention chunks so the
# Tile scheduler can gap-fill TensorE while ScalarE runs the exps (keeps the
# PE HAM clock-gate warm).

import math

import numpy as np

B, T, C, H = 4, 2048, 1024, 16
D = 64
NCORES = 8
HC = 8          # heads per core
CL = HC * D     # 512 local channels
QCH = 512       # q chunk
NTT = T // 128  # 16 t-tiles
NCH = T // QCH  # 4 chunks
SCALE = 1.0 / math.sqrt(D)
EXP_SHIFT = -4.0
MASK_VAL = -1e30

_CACHE = {}


def _build_bass():
    import concourse.tile as tile
    from concourse import bacc, mybir
    from concourse.masks import make_identity

    dt = mybir.dt
    f32 = dt.float32
    bf16 = dt.bfloat16

    nc = bacc.Bacc(None, target_bir_lowering=False)

    x_d = nc.declare_dram_parameter("x", [T, C], f32, isOutput=False)
    wq_d = nc.declare_dram_parameter("wq", [C, CL], f32, isOutput=False)
    wk_d = nc.declare_dram_parameter("wk", [C, CL], f32, isOutput=False)
    wv_d = nc.declare_dram_parameter("wv", [C, CL], f32, isOutput=False)
    wp_d = nc.declare_dram_parameter("wp", [CL, C], f32, isOutput=False)
    bq_d = nc.declare_dram_parameter("bq", [CL], f32, isOutput=False)
    bk_d = nc.declare_dram_parameter("bk", [CL], f32, isOutput=False)
    out_d = nc.declare_dram_parameter("out", [T, C], f32, isOutput=True)

    NCT = C // 128  # 8 c-tiles
    NM = CL // 128  # 4 dloc-tiles

    with tile.TileContext(nc) as tc:
        with (
            tc.tile_pool(name="const", bufs=1) as constp,
            tc.tile_pool(name="persist", bufs=1) as pers,
            tc.tile_pool(name="wpool", bufs=1) as wpool,
            tc.tile_pool(name="xpool", bufs=1) as xpool,
            tc.tile_pool(name="xtc", bufs=16) as xtcp,
            tc.tile_pool(name="ptp", bufs=4) as ptp,
            tc.tile_pool(name="denp", bufs=4) as denp,
            tc.tile_pool(name="bcp", bufs=4) as bcp,
            tc.tile_pool(name="otn", bufs=10) as otnp,
            tc.tile_pool(name="yp", bufs=4) as ypool,
            tc.tile_pool(name="ppool", bufs=2, space="PSUM") as ppool,
            tc.tile_pool(name="stp", bufs=2, space="PSUM") as stp,
            tc.tile_pool(name="otp", bufs=2, space="PSUM") as otp,
        ):
            # ---- constants built on-chip ----
            ident = constp.tile([128, 128], f32, tag="ident")
            make_identity(nc, ident[:])

            # master causal mask [128, 640]: master[i, jj] = 0 if jj >= i+512
            # else MASK_VAL.  Slicing cols [512:640] gives the triangular
            # window mask (valid iff j >= i) applied to each diagonal k-tile.
            mask = constp.tile([128, 640], f32, tag="mask")
            nc.gpsimd.memset(mask[:], 0.0)
            nc.gpsimd.affine_select(
                out=mask[:],
                in_=mask[:],
                compare_op=mybir.AluOpType.is_ge,
                fill=MASK_VAL,
                base=-512,
                pattern=[[1, 640]],
                channel_multiplier=-1,
            )

            eshift = constp.tile([128, 1], f32, tag="eshift")
            nc.gpsimd.memset(eshift[:], EXP_SHIFT)

            bq_sb = constp.tile([128, NM], f32, tag="bq")
            bk_sb = constp.tile([128, NM], f32, tag="bk")
            nc.sync.dma_start(bq_sb[:], bq_d.rearrange("(m p) -> p m", p=128))
            nc.sync.dma_start(bk_sb[:], bk_d.rearrange("(m p) -> p m", p=128))

            # ---- persistent tensors ----
            qt = [pers.tile([128, T], bf16, tag=f"qt{m}", name=f"qt{m}")
                  for m in range(NM)]
            kt = [pers.tile([128, T], bf16, tag=f"kt{m}", name=f"kt{m}")
                  for m in range(NM)]
            vt = [pers.tile([128, HC * 65], bf16, tag=f"vt{t}", name=f"vt{t}")
                  for t in range(NTT)]
            wp8 = [pers.tile([64, C], bf16, tag=f"wp{h}", name=f"wp{h}")
                   for h in range(HC)]
            for h in range(HC):
                wptmp = xpool.tile([64, C], f32, tag="wptmp", bufs=2, name="wptmp")
                nc.sync.dma_start(wptmp[:], wp_d[h * 64:(h + 1) * 64, :])
                nc.vector.tensor_copy(wp8[h][:], wptmp[:])
            for t in range(NTT):
                # ones column per head for the fused denominator row
                nc.gpsimd.memset(
                    vt[t].rearrange("p (h e) -> p h e", e=65)[:, :, 64:65], 1.0
                )

            wq_sb = [wpool.tile([128, CL], bf16, tag=f"wq{c}", name=f"wq{c}")
                     for c in range(NCT)]
            wk_sb = [wpool.tile([128, CL], bf16, tag=f"wk{c}", name=f"wk{c}")
                     for c in range(NCT)]
            wv_sb = [wpool.tile([128, CL], bf16, tag=f"wv{c}", name=f"wv{c}")
                     for c in range(NCT)]
            for c in range(NCT):
                for w_sb, w_d in ((wq_sb, wq_d), (wk_sb, wk_d), (wv_sb, wv_d)):
                    wtmp = xpool.tile([128, CL], f32, tag="wtmp", bufs=2,
                                      name="wtmp")
                    nc.sync.dma_start(wtmp[:], w_d[c * 128:(c + 1) * 128, :])
                    nc.vector.tensor_copy(w_sb[c][:], wtmp[:])

            def phase_a_chunk(tch):
                # load 4 natural x t-tiles, transpose into an xT chunk, then
                # project to QT/KT (chunk of columns) and V (4 t-tiles).
                xns = []
                for tt in range(4):
                    t0 = tch * QCH + tt * 128
                    xn = xpool.tile([128, C], f32, tag="xn", bufs=6, name="xn")
                    nc.sync.dma_start(xn[:], x_d[t0:t0 + 128, :])
                    xns.append(xn)
                xtc = [xtcp.tile([128, QCH], bf16, tag="xtc", name="xtc")
                       for _ in range(NCT)]
                for c in range(NCT):
                    ps = ppool.tile([128, QCH], f32, tag="ps", name="ps_tp")
                    for tt in range(4):
                        nc.tensor.transpose(
                            ps[:, tt * 128:(tt + 1) * 128],
                            xns[tt][:, c * 128:(c + 1) * 128],
                            ident[:],
                        )
                    nc.vector.tensor_copy(xtc[c][:], ps[:])

                for dst, w_sb, b_sb in (
                    (qt, wq_sb, bq_sb),
                    (kt, wk_sb, bk_sb),
                ):
                    for m in range(NM):
                        ps = ppool.tile([128, QCH], f32, tag="ps", name="ps_qk")
                        for c in range(NCT):
                            nc.tensor.matmul(
                                ps[:],
                                w_sb[c][:, m * 128:(m + 1) * 128],
                                xtc[c][:],
                                start=(c == 0),
                                stop=(c == NCT - 1),
                            )
                        nc.scalar.activation(
                            dst[m][:, tch * QCH:(tch + 1) * QCH],
                            ps[:],
                            mybir.ActivationFunctionType.Identity,
                            bias=b_sb[:, m:m + 1],
                            scale=1.0,
                        )

                for tt in range(4):
                    ps = ppool.tile([128, CL], f32, tag="ps", name="ps_v")
                    for c in range(NCT):
                        nc.tensor.matmul(
                            ps[:],
                            xtc[c][:, tt * 128:(tt + 1) * 128],
                            wv_sb[c][:],
                            start=(c == 0),
                            stop=(c == NCT - 1),
                        )
                    vtile = vt[tch * 4 + tt]
                    nc.vector.tensor_copy(
                        vtile.rearrange("p (h e) -> p h e", e=65)[:, :, 0:64],
                        ps.rearrange("p (h d) -> p h d", d=64)[:],
                    )

            def attention_chunk(qc):
                nkt = 4 * (qc + 1)
                otn = []
                for pair in range(HC // 2):
                    h0 = pair * 2
                    mt = h0 // 2
                    ot_ps = [otp.tile([65, QCH], f32, tag="ot", name="ot")
                             for _ in range(2)]
                    for k in range(nkt):
                        stq = stp.tile([128, 2 * QCH], f32, tag="st", name="st")
                        for h2 in range(2):
                            r0 = h2 * 64
                            nc.tensor.matmul(
                                stq[:, h2 * QCH:(h2 + 1) * QCH],
                                kt[mt][r0:r0 + 64, k * 128:(k + 1) * 128],
                                qt[mt][r0:r0 + 64, qc * QCH:(qc + 1) * QCH],
                                start=True,
                                stop=True,
                            )
                        off = (k - 4 * qc) * 128
                        pt = ptp.tile([128, 2 * QCH], bf16, tag="pt", name="pt")
                        st3 = stq.rearrange("p (h w) -> p h w", h=2)
                        pt3 = pt.rearrange("p (h w) -> p h w", h=2)
                        if off >= 0:
                            # diagonal tile: triangular mask on the 128-wide
                            # window, zero the fully-masked left columns, and
                            # run exp only on the live region.
                            for h2 in range(2):
                                nc.vector.tensor_add(
                                    stq[:, h2 * QCH + off:h2 * QCH + off + 128],
                                    stq[:, h2 * QCH + off:h2 * QCH + off + 128],
                                    mask[:, 512:640],
                                )
                            if off > 0:
                                nc.gpsimd.memset(pt3[:, :, 0:off], 0.0)
                            nc.scalar.activation(
                                pt3[:, :, off:QCH],
                                st3[:, :, off:QCH],
                                mybir.ActivationFunctionType.Exp,
                                bias=eshift[:],
                                scale=SCALE,
                            )
                        else:
                            nc.scalar.activation(
                                pt[:],
                                stq[:],
                                mybir.ActivationFunctionType.Exp,
                                bias=eshift[:],
                                scale=SCALE,
                            )
                        for h2 in range(2):
                            nc.tensor.matmul(
                                ot_ps[h2][:],
                                vt[k][:, (h0 + h2) * 65:(h0 + h2 + 1) * 65],
                                pt[:, h2 * QCH:(h2 + 1) * QCH],
                                start=(k == 0),
                                stop=(k == nkt - 1),
                            )
                    # normalize the pair: den lives at psum partition 64; hw
                    # partition_broadcast only reads partition 0, so bounce
                    # the row to partition 0 with a tiny sbuf-to-sbuf DMA.
                    for h2 in range(2):
                        den = denp.tile([65, QCH], f32, tag="den", name="den")
                        nc.vector.tensor_copy(den[64:65, :], ot_ps[h2][64:65, :])
                        den0 = denp.tile([1, QCH], f32, tag="den0", name="den0")
                        nc.sync.dma_start(den0[:], den[64:65, :])
                        rec = denp.tile([1, QCH], f32, tag="rec", name="rec")
                        nc.vector.reciprocal(rec[:], den0[:])
                        bt = bcp.tile([64, QCH], f32, tag="bt", name="bt")
                        nc.gpsimd.partition_broadcast(bt[:], rec[0:1, :])
                        on = otnp.tile([64, QCH], bf16, tag="otn", name="otn")
                        nc.vector.tensor_mul(on[:], ot_ps[h2][0:64, :], bt[:])
                        otn.append(on)

                # projection: y[q, c_out] = sum_h otn_h.T @ wp_h
                for qs in range(4):
                    q0 = qc * QCH + qs * 128
                    for half in range(2):
                        yps = ppool.tile([128, 512], f32, tag="ps", name="ps_y")
                        for h in range(HC):
                            nc.tensor.matmul(
                                yps[:],
                                otn[h][:, qs * 128:(qs + 1) * 128],
                                wp8[h][:, half * 512:(half + 1) * 512],
                                start=(h == 0),
                                stop=(h == HC - 1),
                            )
                        ysb = ypool.tile([128, 512], f32, tag="ysb", name="ysb")
                        nc.vector.tensor_copy(ysb[:], yps[:])
                        nc.sync.dma_start(
                            out_d[q0:q0 + 128, half * 512:(half + 1) * 512],
                            ysb[:],
                        )

            for tch in range(NCH):
                phase_a_chunk(tch)
                attention_chunk(tch)

    return nc


def _get_nc():
    if "nc" not in _CACHE:
        nc = _build_bass()
        nc.compile()
        _CACHE["nc"] = nc
    return _CACHE["nc"]


def _make_in_maps(x, wq, bq, wk, bk, wv, bv, wp, bp):
    in_maps = []
    for core in range(NCORES):
        b, g = core // 2, core % 2
        cs = slice(g * CL, (g + 1) * CL)
        in_maps.append(
            {
                "x": np.ascontiguousarray(x[b], dtype=np.float32),
                "wq": np.ascontiguousarray(wq[:, cs], dtype=np.float32),
                "wk": np.ascontiguousarray(wk[:, cs], dtype=np.float32),
                "wv": np.ascontiguousarray(wv[:, cs], dtype=np.float32),
                "wp": np.ascontiguousarray(wp[cs, :], dtype=np.float32),
                "bq": np.ascontiguousarray(bq[cs], dtype=np.float32),
                "bk": np.ascontiguousarray(bk[cs], dtype=np.float32),
            }
        )
    return in_maps


def kernel(x, wq, bq, wk, bk, wv, bv, wp, bp, _trace=False):
    from concourse.bass_utils import run_bass_kernel_spmd

    x = np.asarray(x, dtype=np.float32)
    nc = _get_nc()
    in_maps = _make_in_maps(x, wq, bq, wk, bk, wv, bv, wp, bp)
    res = run_bass_kernel_spmd(
        nc, in_maps, core_ids=list(range(NCORES)), trace=_trace
    )
    _CACHE["last_results"] = res

    y = np.zeros((B, T, C), dtype=np.float32)
    for core in range(NCORES):
        b = core // 2
        y[b] += res.results[core]["out"]
    # bias terms commute through normalized attention: y += bv @ wp + bp
    const = np.asarray(bv, np.float32) @ np.asarray(wp, np.float32) + np.asarray(
        bp, np.float32
    )
    y += const[None, None, :]
    return y


# revision 11
# speedup vs baseline: 1.2824x; 1.1142x over previous
# Causal self-attention (B=4, T=2048, C=1024, H=16) on 8 TRN2 NeuronCores.
#
# Sharding: core = 2*b + g  (b in 0..3 data-parallel over batch,
# g in 0..1 tensor-parallel over head halves: 8 heads per core).
# Each core gets x[b] and the column slice of Wq/Wk/Wv (cols g*512..) and the
# row slice of Wp (rows g*512..), computes a partial y[b] = attn_g(x[b]) @ Wp_g,
# and the host sums the two partials per batch (the "all-reduce") and adds the
# bias terms (bv @ wp + bp), which commute exactly through softmax-normalized
# attention.
#
# On-chip layout is fully transposed ("channels on partitions"):
#   xT[c, t]  -> QT/KT[d_local, t] (bf16), V[t, d_local] (bf16, +ones col)
#   S^T[k, q] = KT_tile.T @ QT_chunk       (bf16, k on partitions; the two
#              heads of a pair go to the two halves of one [128,1024] psum)
#   P^T = exp(scale*S^T + causal mask)     (one 3D ScalarE exp per pair-tile,
#              no-max softmax with constant -4 shift; masked-left columns
#              skipped and zeroed by a gpsimd memset instead)
#   O^T_aug[65, q] = V_aug.T @ P^T         (row 64 accumulates the denominator)
#   O^T_norm = O^T * bcast(1/den)          (den row -> partition 0 via sbuf
#              DMA, then gpsimd partition_broadcast, then one DVE mul)
#   y[q, c] = sum_h O^T_norm_h.T @ Wp_h    (natural output layout, no final
#              transpose)
#
# Projection work for t-chunk i+1 is emitted between attention chunks so the
# Tile scheduler can gap-fill TensorE while ScalarE runs the exps (keeps the
# PE HAM clock-gate warm).

import math

import numpy as np

B, T, C, H = 4, 2048, 1024, 16
D = 64
NCORES = 8
HC = 8          # heads per core
CL = HC * D     # 512 local channels
QCH = 512       # q chunk
NTT = T // 128  # 16 t-tiles
NCH = T // QCH  # 4 chunks
SCALE = 1.0 / math.sqrt(D)
EXP_SHIFT = -4.0
MASK_VAL = -1e30

_CACHE = {}


def _build_bass():
    import concourse.tile as tile
    from concourse import bacc, mybir
    from concourse.masks import make_identity

    dt = mybir.dt
    f32 = dt.float32
    bf16 = dt.bfloat16

    nc = bacc.Bacc(None, target_bir_lowering=False)

    x_d = nc.declare_dram_parameter("x", [T, C], f32, isOutput=False)
    wq_d = nc.declare_dram_parameter("wq", [C, CL], f32, isOutput=False)
    wk_d = nc.declare_dram_parameter("wk", [C, CL], f32, isOutput=False)
    wv_d = nc.declare_dram_parameter("wv", [C, CL], f32, isOutput=False)
    wp_d = nc.declare_dram_parameter("wp", [CL, C], f32, isOutput=False)
    bq_d = nc.declare_dram_parameter("bq", [CL], f32, isOutput=False)
    bk_d = nc.declare_dram_parameter("bk", [CL], f32, isOutput=False)
    out_d = nc.declare_dram_parameter("out", [T, C], f32, isOutput=True)

    NCT = C // 128  # 8 c-tiles
    NM = CL // 128  # 4 dloc-tiles

    with tile.TileContext(nc) as tc:
        with (
            tc.tile_pool(name="const", bufs=1) as constp,
            tc.tile_pool(name="persist", bufs=1) as pers,
            tc.tile_pool(name="wpool", bufs=1) as wpool,
            tc.tile_pool(name="xpool", bufs=1) as xpool,
            tc.tile_pool(name="xtc", bufs=16) as xtcp,
            tc.tile_pool(name="ptp", bufs=4) as ptp,
            tc.tile_pool(name="denp", bufs=2) as denp,
            tc.tile_pool(name="otsb", bufs=10) as otsbp,
            tc.tile_pool(name="otn", bufs=10) as otnp,
            tc.tile_pool(name="yp", bufs=4) as ypool,
            tc.tile_pool(name="ppool", bufs=2, space="PSUM") as ppool,
            tc.tile_pool(name="stp", bufs=2, space="PSUM") as stp,
            tc.tile_pool(name="otp", bufs=2, space="PSUM") as otp,
        ):
            # ---- constants built on-chip ----
            ident = constp.tile([128, 128], f32, tag="ident")
            make_identity(nc, ident[:])

            # master causal mask [128, 640]: master[i, jj] = 0 if jj >= i+512
            # else MASK_VAL.  Slicing cols [512:640] gives the triangular
            # window mask (valid iff j >= i) applied to each diagonal k-tile.
            mask = constp.tile([128, 640], f32, tag="mask")
            nc.gpsimd.memset(mask[:], 0.0)
            nc.gpsimd.affine_select(
                out=mask[:],
                in_=mask[:],
                compare_op=mybir.AluOpType.is_ge,
                fill=MASK_VAL,
                base=-512,
                pattern=[[1, 640]],
                channel_multiplier=-1,
            )

            # E8[j, h*64+p] = (j == h): broadcast selector for the
            # per-head 1/den rows; bt_h = E8[:, h*64:(h+1)*64].T @ recip_all
            e8 = constp.tile([8, HC * 64], bf16, tag="e8")
            nc.gpsimd.memset(e8[:], 1.0)
            nc.gpsimd.affine_select(
                out=e8[:], in_=e8[:], compare_op=mybir.AluOpType.is_ge,
                fill=0.0, base=0, pattern=[[1, HC * 64]], channel_multiplier=-64,
            )
            nc.gpsimd.affine_select(
                out=e8[:], in_=e8[:], compare_op=mybir.AluOpType.is_ge,
                fill=0.0, base=63, pattern=[[-1, HC * 64]], channel_multiplier=64,
            )

            eshift = constp.tile([128, 1], f32, tag="eshift")
            nc.gpsimd.memset(eshift[:], EXP_SHIFT)

            bq_sb = constp.tile([128, NM], f32, tag="bq")
            bk_sb = constp.tile([128, NM], f32, tag="bk")
            nc.sync.dma_start(bq_sb[:], bq_d.rearrange("(m p) -> p m", p=128))
            nc.sync.dma_start(bk_sb[:], bk_d.rearrange("(m p) -> p m", p=128))

            # ---- persistent tensors ----
            qt = [pers.tile([128, T], bf16, tag=f"qt{m}", name=f"qt{m}")
                  for m in range(NM)]
            kt = [pers.tile([128, T], bf16, tag=f"kt{m}", name=f"kt{m}")
                  for m in range(NM)]
            vt = [pers.tile([128, HC * 65], bf16, tag=f"vt{t}", name=f"vt{t}")
                  for t in range(NTT)]
            wp8 = [pers.tile([64, C], bf16, tag=f"wp{h}", name=f"wp{h}")
                   for h in range(HC)]
            for h in range(HC):
                wptmp = xpool.tile([64, C], f32, tag="wptmp", bufs=2, name="wptmp")
                nc.sync.dma_start(wptmp[:], wp_d[h * 64:(h + 1) * 64, :])
                nc.vector.tensor_copy(wp8[h][:], wptmp[:])
            for t in range(NTT):
                # ones column per head for the fused denominator row
                nc.gpsimd.memset(
                    vt[t].rearrange("p (h e) -> p h e", e=65)[:, :, 64:65], 1.0
                )

            wq_sb = [wpool.tile([128, CL], bf16, tag=f"wq{c}", name=f"wq{c}")
                     for c in range(NCT)]
            wk_sb = [wpool.tile([128, CL], bf16, tag=f"wk{c}", name=f"wk{c}")
                     for c in range(NCT)]
            wv_sb = [wpool.tile([128, CL], bf16, tag=f"wv{c}", name=f"wv{c}")
                     for c in range(NCT)]
            for c in range(NCT):
                for w_sb, w_d in ((wq_sb, wq_d), (wk_sb, wk_d), (wv_sb, wv_d)):
                    wtmp = xpool.tile([128, CL], f32, tag="wtmp", bufs=2,
                                      name="wtmp")
                    nc.sync.dma_start(wtmp[:], w_d[c * 128:(c + 1) * 128, :])
                    nc.vector.tensor_copy(w_sb[c][:], wtmp[:])

            def phase_a_chunk(tch):
                # load 4 natural x t-tiles, transpose into an xT chunk, then
                # project to QT/KT (chunk of columns) and V (4 t-tiles).
                xns = []
                for tt in range(4):
                    t0 = tch * QCH + tt * 128
                    xn = xpool.tile([128, C], f32, tag="xn", bufs=6, name="xn")
                    nc.sync.dma_start(xn[:], x_d[t0:t0 + 128, :])
                    xns.append(xn)
                xtc = [xtcp.tile([128, QCH], bf16, tag="xtc", name="xtc")
                       for _ in range(NCT)]
                for c in range(NCT):
                    ps = ppool.tile([128, QCH], f32, tag="ps", name="ps_tp")
                    for tt in range(4):
                        nc.tensor.transpose(
                            ps[:, tt * 128:(tt + 1) * 128],
                            xns[tt][:, c * 128:(c + 1) * 128],
                            ident[:],
                        )
                    nc.vector.tensor_copy(xtc[c][:], ps[:])

                for dst, w_sb, b_sb in (
                    (qt, wq_sb, bq_sb),
                    (kt, wk_sb, bk_sb),
                ):
                    for m in range(NM):
                        ps = ppool.tile([128, QCH], f32, tag="ps", name="ps_qk")
                        for c in range(NCT):
                            nc.tensor.matmul(
                                ps[:],
                                w_sb[c][:, m * 128:(m + 1) * 128],
                                xtc[c][:],
                                start=(c == 0),
                                stop=(c == NCT - 1),
                            )
                        nc.scalar.activation(
                            dst[m][:, tch * QCH:(tch + 1) * QCH],
                            ps[:],
                            mybir.ActivationFunctionType.Identity,
                            bias=b_sb[:, m:m + 1],
                            scale=1.0,
                        )

                for tt in range(4):
                    ps = ppool.tile([128, CL], f32, tag="ps", name="ps_v")
                    for c in range(NCT):
                        nc.tensor.matmul(
                            ps[:],
                            xtc[c][:, tt * 128:(tt + 1) * 128],
                            wv_sb[c][:],
                            start=(c == 0),
                            stop=(c == NCT - 1),
                        )
                    vtile = vt[tch * 4 + tt]
                    nc.vector.tensor_copy(
                        vtile.rearrange("p (h e) -> p h e", e=65)[:, :, 0:64],
                        ps.rearrange("p (h d) -> p h d", d=64)[:],
                    )

            def attention_chunk(qc):
                nkt = 4 * (qc + 1)
                ot_sb = []
                den_all = denp.tile([HC, QCH], f32, tag="den", name="den")
                for pair in range(HC // 2):
                    h0 = pair * 2
                    mt = h0 // 2
                    ot_ps = [otp.tile([65, QCH], f32, tag="ot", name="ot")
                             for _ in range(2)]
                    for k in range(nkt):
                        stq = stp.tile([128, 2 * QCH], f32, tag="st", name="st")
                        for h2 in range(2):
                            r0 = h2 * 64
                            nc.tensor.matmul(
                                stq[:, h2 * QCH:(h2 + 1) * QCH],
                                kt[mt][r0:r0 + 64, k * 128:(k + 1) * 128],
                                qt[mt][r0:r0 + 64, qc * QCH:(qc + 1) * QCH],
                                start=True,
                                stop=True,
                            )
                        off = (k - 4 * qc) * 128
                        pt = ptp.tile([128, 2 * QCH], bf16, tag="pt", name="pt")
                        st3 = stq.rearrange("p (h w) -> p h w", h=2)
                        pt3 = pt.rearrange("p (h w) -> p h w", h=2)
                        if off >= 0:
                            # diagonal tile: triangular mask on the 128-wide
                            # window, zero the fully-masked left columns, and
                            # run exp only on the live region.
                            for h2 in range(2):
                                nc.vector.tensor_add(
                                    stq[:, h2 * QCH + off:h2 * QCH + off + 128],
                                    stq[:, h2 * QCH + off:h2 * QCH + off + 128],
                                    mask[:, 512:640],
                                )
                            if off > 0:
                                nc.gpsimd.memset(pt3[:, :, 0:off], 0.0)
                            nc.scalar.activation(
                                pt3[:, :, off:QCH],
                                st3[:, :, off:QCH],
                                mybir.ActivationFunctionType.Exp,
                                bias=eshift[:],
                                scale=SCALE,
                            )
                        else:
                            nc.scalar.activation(
                                pt[:],
                                stq[:],
                                mybir.ActivationFunctionType.Exp,
                                bias=eshift[:],
                                scale=SCALE,
                            )
                        for h2 in range(2):
                            nc.tensor.matmul(
                                ot_ps[h2][:],
                                vt[k][:, (h0 + h2) * 65:(h0 + h2 + 1) * 65],
                                pt[:, h2 * QCH:(h2 + 1) * QCH],
                                start=(k == 0),
                                stop=(k == nkt - 1),
                            )
                    # move the pair's O^T (+den row) to SBUF right away so
                    # the psum banks free up for the next pair.
                    for h2 in range(2):
                        osb = otsbp.tile([65, QCH], f32, tag="otsb", name="otsb")
                        nc.vector.tensor_copy(osb[:], ot_ps[h2][:])
                        nc.sync.dma_start(
                            den_all[h0 + h2:h0 + h2 + 1, :], osb[64:65, :]
                        )
                        ot_sb.append(osb)

                # batched reciprocal of all 8 denominators, then broadcast
                # each row with a tiny E8 matmul and normalize.
                rec = denp.tile([HC, QCH], f32, tag="rec", name="rec")
                nc.vector.reciprocal(rec[:], den_all[:])
                recbf = denp.tile([HC, QCH], bf16, tag="recbf", name="recbf")
                nc.vector.tensor_copy(recbf[:], rec[:])
                otn = []
                for h in range(HC):
                    btp = ppool.tile([64, QCH], f32, tag="ps", name="ps_bt")
                    nc.tensor.matmul(
                        btp[:], e8[:, h * 64:(h + 1) * 64], recbf[:],
                        start=True, stop=True,
                    )
                    on = otnp.tile([64, QCH], bf16, tag="otn", name="otn")
                    nc.vector.tensor_mul(on[:], ot_sb[h][0:64, :], btp[:])
                    otn.append(on)

                # projection: y[q, c_out] = sum_h otn_h.T @ wp_h
                for qs in range(4):
                    q0 = qc * QCH + qs * 128
                    for half in range(2):
                        yps = ppool.tile([128, 512], f32, tag="ps", name="ps_y")
                        for h in range(HC):
                            nc.tensor.matmul(
                                yps[:],
                                otn[h][:, qs * 128:(qs + 1) * 128],
                                wp8[h][:, half * 512:(half + 1) * 512],
                                start=(h == 0),
                                stop=(h == HC - 1),
                            )
                        ysb = ypool.tile([128, 512], f32, tag="ysb", name="ysb")
                        nc.vector.tensor_copy(ysb[:], yps[:])
                        nc.sync.dma_start(
                            out_d[q0:q0 + 128, half * 512:(half + 1) * 512],
                            ysb[:],
                        )

            for tch in range(NCH):
                phase_a_chunk(tch)
                attention_chunk(tch)

    return nc


def _get_nc():
    if "nc" not in _CACHE:
        nc = _build_bass()
        nc.compile()
        _CACHE["nc"] = nc
    return _CACHE["nc"]


def _make_in_maps(x, wq, bq, wk, bk, wv, bv, wp, bp):
    in_maps = []
    for core in range(NCORES):
        b, g = core // 2, core % 2
        cs = slice(g * CL, (g + 1) * CL)
        in_maps.append(
            {
                "x": np.ascontiguousarray(x[b], dtype=np.float32),
                "wq": np.ascontiguousarray(wq[:, cs], dtype=np.float32),
                "wk": np.ascontiguousarray(wk[:, cs], dtype=np.float32),
                "wv": np.ascontiguousarray(wv[:, cs], dtype=np.float32),
                "wp": np.ascontiguousarray(wp[cs, :], dtype=np.float32),
                "bq": np.ascontiguousarray(bq[cs], dtype=np.float32),
                "bk": np.ascontiguousarray(bk[cs], dtype=np.float32),
            }
        )
    return in_maps


def kernel(x, wq, bq, wk, bk, wv, bv, wp, bp, _trace=False):
    from concourse.bass_utils import run_bass_kernel_spmd

    x = np.asarray(x, dtype=np.float32)
    nc = _get_nc()
    in_maps = _make_in_maps(x, wq, bq, wk, bk, wv, bv, wp, bp)
    res = run_bass_kernel_spmd(
        nc, in_maps, core_ids=list(range(NCORES)), trace=_trace
    )
    _CACHE["last_results"] = res

    y = np.zeros((B, T, C), dtype=np.float32)
    for core in range(NCORES):
        b = core // 2
        y[b] += res.results[core]["out"]
    # bias terms commute through normalized attention: y += bv @ wp + bp
    const = np.asarray(bv, np.float32) @ np.asarray(wp, np.float32) + np.asarray(
        bp, np.float32
    )
    y += const[None, None, :]
    return y


# revision 12
# speedup vs baseline: 1.4808x; 1.1547x over previous
# Causal self-attention (B=4, T=2048, C=1024, H=16) on 8 TRN2 NeuronCores.
#
# Sharding: core = 2*b + g  (b in 0..3 data-parallel over batch,
# g in 0..1 tensor-parallel over head halves: 8 heads per core).
# Each core gets x[b] and the column slice of Wq/Wk/Wv (cols g*512..) and the
# row slice of Wp (rows g*512..), computes a partial y[b] = attn_g(x[b]) @ Wp_g,
# and the host sums the two partials per batch (the "all-reduce") and adds the
# bias terms (bv @ wp + bp), which commute exactly through softmax-normalized
# attention.
#
# On-chip layout is fully transposed ("channels on partitions"):
#   xT[c, t]  -> QT/KT[d_local, t] (bf16), V[t, d_local] (bf16, +ones col)
#   S^T[k, q] = KT_tile.T @ QT_chunk       (bf16, k on partitions; the two
#              heads of a pair go to the two halves of one [128,1024] psum)
#   P^T = exp(scale*S^T + causal mask)     (one 3D ScalarE exp per pair-tile,
#              no-max softmax with constant -4 shift; masked-left columns
#              skipped and zeroed by a gpsimd memset instead)
#   O^T_aug[65, q] = V_aug.T @ P^T         (row 64 accumulates the denominator)
#   O^T_norm = O^T * bcast(1/den)          (den row -> partition 0 via sbuf
#              DMA, then gpsimd partition_broadcast, then one DVE mul)
#   y[q, c] = sum_h O^T_norm_h.T @ Wp_h    (natural output layout, no final
#              transpose)
#
# Projection work for t-chunk i+1 is emitted between attention chunks so the
# Tile scheduler can gap-fill TensorE while ScalarE runs the exps (keeps the
# PE HAM clock-gate warm).

import math

import numpy as np

B, T, C, H = 4, 2048, 1024, 16
D = 64
NCORES = 8
HC = 8          # heads per core
CL = HC * D     # 512 local channels
QCH = 512       # q chunk
NTT = T // 128  # 16 t-tiles
NCH = T // QCH  # 4 chunks
SCALE = 1.0 / math.sqrt(D)
EXP_SHIFT = -4.0
MASK_VAL = -1e30

_CACHE = {}


def _build_bass():
    import concourse.tile as tile
    from concourse import bacc, mybir
    from concourse.masks import make_identity

    dt = mybir.dt
    f32 = dt.float32
    bf16 = dt.bfloat16

    nc = bacc.Bacc(None, target_bir_lowering=False)

    x_d = nc.declare_dram_parameter("x", [T, C], f32, isOutput=False)
    wq_d = nc.declare_dram_parameter("wq", [C, CL], f32, isOutput=False)
    wk_d = nc.declare_dram_parameter("wk", [C, CL], f32, isOutput=False)
    wv_d = nc.declare_dram_parameter("wv", [C, CL], f32, isOutput=False)
    wp_d = nc.declare_dram_parameter("wp", [CL, C], f32, isOutput=False)
    bq_d = nc.declare_dram_parameter("bq", [CL], f32, isOutput=False)
    bk_d = nc.declare_dram_parameter("bk", [CL], f32, isOutput=False)
    out_d = nc.declare_dram_parameter("out", [T, C], f32, isOutput=True)

    NCT = C // 128  # 8 c-tiles
    NM = CL // 128  # 4 dloc-tiles

    with tile.TileContext(nc) as tc:
        with (
            tc.tile_pool(name="const", bufs=1) as constp,
            tc.tile_pool(name="persist", bufs=1) as pers,
            tc.tile_pool(name="wpool", bufs=1) as wpool,
            tc.tile_pool(name="xpool", bufs=1) as xpool,
            tc.tile_pool(name="xtc", bufs=16) as xtcp,
            tc.tile_pool(name="ptp", bufs=4) as ptp,
            tc.tile_pool(name="denp", bufs=2) as denp,
            tc.tile_pool(name="otsb", bufs=10) as otsbp,
            tc.tile_pool(name="otn", bufs=10) as otnp,
            tc.tile_pool(name="yp", bufs=4) as ypool,
            tc.tile_pool(name="ppool", bufs=2, space="PSUM") as ppool,
            tc.tile_pool(name="stp", bufs=2, space="PSUM") as stp,
            tc.tile_pool(name="otp", bufs=2, space="PSUM") as otp,
        ):
            # ---- constants built on-chip ----
            ident = constp.tile([128, 128], f32, tag="ident")
            make_identity(nc, ident[:])

            # master causal mask [128, 640]: master[i, jj] = 0 if jj >= i+512
            # else MASK_VAL.  Slicing cols [512:640] gives the triangular
            # window mask (valid iff j >= i) applied to each diagonal k-tile.
            mask = constp.tile([128, 640], f32, tag="mask")
            nc.gpsimd.memset(mask[:], 0.0)
            nc.gpsimd.affine_select(
                out=mask[:],
                in_=mask[:],
                compare_op=mybir.AluOpType.is_ge,
                fill=MASK_VAL,
                base=-512,
                pattern=[[1, 640]],
                channel_multiplier=-1,
            )

            # E8[j, h*64+p] = (j == h): broadcast selector for the
            # per-head 1/den rows; bt_h = E8[:, h*64:(h+1)*64].T @ recip_all
            e8 = constp.tile([8, HC * 64], bf16, tag="e8")
            nc.gpsimd.memset(e8[:], 1.0)
            nc.gpsimd.affine_select(
                out=e8[:], in_=e8[:], compare_op=mybir.AluOpType.is_ge,
                fill=0.0, base=0, pattern=[[1, HC * 64]], channel_multiplier=-64,
            )
            nc.gpsimd.affine_select(
                out=e8[:], in_=e8[:], compare_op=mybir.AluOpType.is_ge,
                fill=0.0, base=63, pattern=[[-1, HC * 64]], channel_multiplier=64,
            )

            eshift = constp.tile([128, 1], f32, tag="eshift")
            nc.gpsimd.memset(eshift[:], EXP_SHIFT)

            bq_sb = constp.tile([128, NM], f32, tag="bq")
            bk_sb = constp.tile([128, NM], f32, tag="bk")
            nc.sync.dma_start(bq_sb[:], bq_d.rearrange("(m p) -> p m", p=128))
            nc.sync.dma_start(bk_sb[:], bk_d.rearrange("(m p) -> p m", p=128))

            # ---- persistent tensors ----
            qt = [pers.tile([128, T], bf16, tag=f"qt{m}", name=f"qt{m}")
                  for m in range(NM)]
            kt = [pers.tile([128, T], bf16, tag=f"kt{m}", name=f"kt{m}")
                  for m in range(NM)]
            vt = [pers.tile([128, HC * 65], bf16, tag=f"vt{t}", name=f"vt{t}")
                  for t in range(NTT)]
            wp4 = [pers.tile([128, C], bf16, tag=f"wp{p}", name=f"wp{p}")
                   for p in range(HC // 2)]
            for p in range(HC // 2):
                wptmp = xpool.tile([128, C], f32, tag="wptmp", bufs=2,
                                   name="wptmp")
                nc.sync.dma_start(wptmp[:], wp_d[p * 128:(p + 1) * 128, :])
                nc.vector.tensor_copy(wp4[p][:], wptmp[:])
            for t in range(NTT):
                # ones column per head for the fused denominator row
                nc.gpsimd.memset(
                    vt[t].rearrange("p (h e) -> p h e", e=65)[:, :, 64:65], 1.0
                )

            wq_sb = [wpool.tile([128, CL], bf16, tag=f"wq{c}", name=f"wq{c}")
                     for c in range(NCT)]
            wk_sb = [wpool.tile([128, CL], bf16, tag=f"wk{c}", name=f"wk{c}")
                     for c in range(NCT)]
            wv_sb = [wpool.tile([128, CL], bf16, tag=f"wv{c}", name=f"wv{c}")
                     for c in range(NCT)]
            for c in range(NCT):
                for w_sb, w_d in ((wq_sb, wq_d), (wk_sb, wk_d), (wv_sb, wv_d)):
                    wtmp = xpool.tile([128, CL], f32, tag="wtmp", bufs=2,
                                      name="wtmp")
                    nc.sync.dma_start(wtmp[:], w_d[c * 128:(c + 1) * 128, :])
                    nc.vector.tensor_copy(w_sb[c][:], wtmp[:])

            def phase_a_chunk(tch):
                # load 4 natural x t-tiles, transpose into an xT chunk, then
                # project to QT/KT (chunk of columns) and V (4 t-tiles).
                xns = []
                for tt in range(4):
                    t0 = tch * QCH + tt * 128
                    xn = xpool.tile([128, C], f32, tag="xn", bufs=6, name="xn")
                    nc.sync.dma_start(xn[:], x_d[t0:t0 + 128, :])
                    xns.append(xn)
                xtc = [xtcp.tile([128, QCH], bf16, tag="xtc", name="xtc")
                       for _ in range(NCT)]
                for c in range(NCT):
                    ps = ppool.tile([128, QCH], f32, tag="ps", name="ps_tp")
                    for tt in range(4):
                        nc.tensor.transpose(
                            ps[:, tt * 128:(tt + 1) * 128],
                            xns[tt][:, c * 128:(c + 1) * 128],
                            ident[:],
                        )
                    nc.vector.tensor_copy(xtc[c][:], ps[:])

                for dst, w_sb, b_sb in (
                    (qt, wq_sb, bq_sb),
                    (kt, wk_sb, bk_sb),
                ):
                    for m in range(NM):
                        ps = ppool.tile([128, QCH], f32, tag="ps", name="ps_qk")
                        for c in range(NCT):
                            nc.tensor.matmul(
                                ps[:],
                                w_sb[c][:, m * 128:(m + 1) * 128],
                                xtc[c][:],
                                start=(c == 0),
                                stop=(c == NCT - 1),
                            )
                        nc.scalar.activation(
                            dst[m][:, tch * QCH:(tch + 1) * QCH],
                            ps[:],
                            mybir.ActivationFunctionType.Identity,
                            bias=b_sb[:, m:m + 1],
                            scale=1.0,
                        )

                for tt in range(4):
                    ps = ppool.tile([128, CL], f32, tag="ps", name="ps_v")
                    for c in range(NCT):
                        nc.tensor.matmul(
                            ps[:],
                            xtc[c][:, tt * 128:(tt + 1) * 128],
                            wv_sb[c][:],
                            start=(c == 0),
                            stop=(c == NCT - 1),
                        )
                    vtile = vt[tch * 4 + tt]
                    nc.vector.tensor_copy(
                        vtile.rearrange("p (h e) -> p h e", e=65)[:, :, 0:64],
                        ps.rearrange("p (h d) -> p h d", d=64)[:],
                    )

            def attention_chunk(qc):
                nkt = 4 * (qc + 1)
                ot_sb = []
                den_all = denp.tile([HC, QCH], f32, tag="den", name="den")
                for pair in range(HC // 2):
                    h0 = pair * 2
                    mt = h0 // 2
                    ot_ps = [otp.tile([65, QCH], f32, tag="ot", name="ot")
                             for _ in range(2)]
                    for k in range(nkt):
                        stq = stp.tile([128, 2 * QCH], f32, tag="st", name="st")
                        for h2 in range(2):
                            r0 = h2 * 64
                            nc.tensor.matmul(
                                stq[:, h2 * QCH:(h2 + 1) * QCH],
                                kt[mt][r0:r0 + 64, k * 128:(k + 1) * 128],
                                qt[mt][r0:r0 + 64, qc * QCH:(qc + 1) * QCH],
                                start=True,
                                stop=True,
                            )
                        off = (k - 4 * qc) * 128
                        pt = ptp.tile([128, 2 * QCH], bf16, tag="pt", name="pt")
                        st3 = stq.rearrange("p (h w) -> p h w", h=2)
                        pt3 = pt.rearrange("p (h w) -> p h w", h=2)
                        if off >= 0:
                            # diagonal tile: triangular mask on the 128-wide
                            # window, zero the fully-masked left columns, and
                            # run exp only on the live region.
                            for h2 in range(2):
                                nc.vector.tensor_add(
                                    stq[:, h2 * QCH + off:h2 * QCH + off + 128],
                                    stq[:, h2 * QCH + off:h2 * QCH + off + 128],
                                    mask[:, 512:640],
                                )
                            if off > 0:
                                nc.gpsimd.memset(pt3[:, :, 0:off], 0.0)
                            nc.scalar.activation(
                                pt3[:, :, off:QCH],
                                st3[:, :, off:QCH],
                                mybir.ActivationFunctionType.Exp,
                                bias=eshift[:],
                                scale=SCALE,
                            )
                        else:
                            nc.scalar.activation(
                                pt[:],
                                stq[:],
                                mybir.ActivationFunctionType.Exp,
                                bias=eshift[:],
                                scale=SCALE,
                            )
                        for h2 in range(2):
                            nc.tensor.matmul(
                                ot_ps[h2][:],
                                vt[k][:, (h0 + h2) * 65:(h0 + h2 + 1) * 65],
                                pt[:, h2 * QCH:(h2 + 1) * QCH],
                                start=(k == 0),
                                stop=(k == nkt - 1),
                            )
                    # move the pair's O^T (+den row) to SBUF right away so
                    # the psum banks free up for the next pair.
                    for h2 in range(2):
                        osb = otsbp.tile([65, QCH], f32, tag="otsb", name="otsb")
                        nc.vector.tensor_copy(osb[:], ot_ps[h2][:])
                        nc.sync.dma_start(
                            den_all[h0 + h2:h0 + h2 + 1, :], osb[64:65, :]
                        )
                        ot_sb.append(osb)

                # batched reciprocal of all 8 denominators, then broadcast
                # each row with a tiny E8 matmul and normalize.
                rec = denp.tile([HC, QCH], f32, tag="rec", name="rec")
                nc.vector.reciprocal(rec[:], den_all[:])
                recbf = denp.tile([HC, QCH], bf16, tag="recbf", name="recbf")
                nc.vector.tensor_copy(recbf[:], rec[:])
                # normalize into pair-stacked tiles: even head lands on
                # partitions 0-63 directly; the odd head is normalized into a
                # scratch tile and DMA'd across partitions to rows 64-127 so
                # the projection can contract K=128 per pair.
                otn = []
                for pair in range(HC // 2):
                    opair = otnp.tile([128, QCH], bf16, tag="otn", name="otn")
                    for h2 in range(2):
                        h = pair * 2 + h2
                        btp = ppool.tile([64, QCH], f32, tag="ps", name="ps_bt")
                        nc.tensor.matmul(
                            btp[:], e8[:, h * 64:(h + 1) * 64], recbf[:],
                            start=True, stop=True,
                        )
                        if h2 == 0:
                            nc.vector.tensor_mul(
                                opair[0:64, :], ot_sb[h][0:64, :], btp[:]
                            )
                        else:
                            oodd = otnp.tile([64, QCH], bf16, tag="oodd",
                                             bufs=4, name="oodd")
                            nc.vector.tensor_mul(
                                oodd[:], ot_sb[h][0:64, :], btp[:]
                            )
                            nc.sync.dma_start(opair[64:128, :], oodd[:])
                    otn.append(opair)

                # projection: y[q, c_out] = sum_h otn_h.T @ wp_h
                for qs in range(4):
                    q0 = qc * QCH + qs * 128
                    for half in range(2):
                        yps = ppool.tile([128, 512], f32, tag="ps", name="ps_y")
                        for p in range(HC // 2):
                            nc.tensor.matmul(
                                yps[:],
                                otn[p][:, qs * 128:(qs + 1) * 128],
                                wp4[p][:, half * 512:(half + 1) * 512],
                                start=(p == 0),
                                stop=(p == HC // 2 - 1),
                            )
                        ysb = ypool.tile([128, 512], f32, tag="ysb", name="ysb")
                        nc.vector.tensor_copy(ysb[:], yps[:])
                        nc.sync.dma_start(
                            out_d[q0:q0 + 128, half * 512:(half + 1) * 512],
                            ysb[:],
                        )

            for tch in range(NCH):
                phase_a_chunk(tch)
                attention_chunk(tch)

    return nc


def _get_nc():
    if "nc" not in _CACHE:
        nc = _build_bass()
        nc.compile()
        _CACHE["nc"] = nc
    return _CACHE["nc"]


def _make_in_maps(x, wq, bq, wk, bk, wv, bv, wp, bp):
    in_maps = []
    for core in range(NCORES):
        b, g = core // 2, core % 2
        cs = slice(g * CL, (g + 1) * CL)
        in_maps.append(
            {
                "x": np.ascontiguousarray(x[b], dtype=np.float32),
                "wq": np.ascontiguousarray(wq[:, cs], dtype=np.float32),
                "wk": np.ascontiguousarray(wk[:, cs], dtype=np.float32),
                "wv": np.ascontiguousarray(wv[:, cs], dtype=np.float32),
                "wp": np.ascontiguousarray(wp[cs, :], dtype=np.float32),
                "bq": np.ascontiguousarray(bq[cs], dtype=np.float32),
                "bk": np.ascontiguousarray(bk[cs], dtype=np.float32),
            }
        )
    return in_maps


def kernel(x, wq, bq, wk, bk, wv, bv, wp, bp, _trace=False):
    from concourse.bass_utils import run_bass_kernel_spmd

    x = np.asarray(x, dtype=np.float32)
    nc = _get_nc()
    in_maps = _make_in_maps(x, wq, bq, wk, bk, wv, bv, wp, bp)
    res = run_bass_kernel_spmd(
        nc, in_maps, core_ids=list(range(NCORES)), trace=_trace
    )
    _CACHE["last_results"] = res

    y = np.zeros((B, T, C), dtype=np.float32)
    for core in range(NCORES):
        b = core // 2
        y[b] += res.results[core]["out"]
    # bias terms commute through normalized attention: y += bv @ wp + bp
    const = np.asarray(bv, np.float32) @ np.asarray(wp, np.float32) + np.asarray(
        bp, np.float32
    )
    y += const[None, None, :]
    return y
